# revision 1
# baseline (speedup 1.0000x reference)
"""Neural ODE layer (3-layer tanh MLP dynamics, RK4, 10 steps) on 8 trn2 cores.

Strategy: data-parallel over batch (8192/8 = 1024 rows per core), weights
replicated (no cross-device communication). Inside each core the batch is
split into 2 chunks of 512 columns, both SBUF-resident and interleaved at
layer granularity (while one chunk's PSUM drains on ACT/DVE, the PE
streams the other chunk's matmuls). All activations live in SBUF
transposed ([hid on partitions, batch free]) so every matmul is
out^T = W^T @ x^T with the weight slice stationary and the activation
moving -- the output lands in exactly the layout the next layer needs, so
the whole 120-matmul chain runs without a single transpose. Matmul
operands (weights, activations) are fp16 -- 1 PE cycle/row with the
weight load hidden by fast-weight-load; the integration state h and the
RK4 accumulator stay fp32. The t-input is folded into per-eval bias vectors
(concat(h,t) @ W1 == h @ W1[:-1] + t*W1[-1]), and the RK4 combine
(h + c*k accumulation) is fused into the PSUM-drain ops on ACT/DVE.

Built as bacc.Bacc and finished with nc.compile(): that pass splits
multi-semaphore waits into EventSemaphore instructions (TRN2 allows one
sync wait per instruction) -- without it walrus codegen rejects any
cross-engine Tile kernel.
"""

import sys

sys.path.insert(0, "/opt/trn_rl_repo")

import numpy as np
from contextlib import ExitStack

import concourse.bacc as bacc
import concourse.tile as tile
from concourse import mybir
from concourse.bass_utils import run_bass_kernel_spmd

HID = 1024
BATCH = 8192
N_CORES = 8
CORE_BATCH = BATCH // N_CORES  # 1024
DT = 0.1
STEPS = 10
P = 128
KT = HID // P  # 8 contraction tiles
MT = HID // P  # 8 output tiles
NCHUNK = 512   # batch columns per chunk (= one fp32 PSUM bank)
CHUNKS = CORE_BATCH // NCHUNK  # 2

F32 = mybir.dt.float32
FP16 = mybir.dt.float16  # same PE speed as bf16, 8x the mantissa
AF = mybir.ActivationFunctionType
ALU = mybir.AluOpType

# RK4: h' = h + dt/6*(k1 + 2k2 + 2k3 + k4)
ACC_W = [DT / 6, DT / 3, DT / 3, DT / 6]   # weight of k_e in the combine
STEP_C = [DT / 2, DT / 2, DT]              # h_tmp = h + c*k_e  (evals 0..2)
T_OFF = [0, 1, 1, 2]                       # t index offset (in dt/2 units)


def build_nc(steps=STEPS, chunks=CHUNKS, reps=1):
    nc = bacc.Bacc("TRN2", target_bir_lowering=False, debug=False)

    h_in = nc.dram_tensor("h", [CORE_BATCH, HID], F32, kind="ExternalInput").ap()
    W1 = nc.dram_tensor("W1", [HID + 1, HID], FP16, kind="ExternalInput").ap()
    b1 = nc.dram_tensor("b1", [HID], F32, kind="ExternalInput").ap()
    W2 = nc.dram_tensor("W2", [HID, HID], FP16, kind="ExternalInput").ap()
    b2 = nc.dram_tensor("b2", [HID], F32, kind="ExternalInput").ap()
    W3 = nc.dram_tensor("W3", [HID, HID], FP16, kind="ExternalInput").ap()
    b3 = nc.dram_tensor("b3", [HID], F32, kind="ExternalInput").ap()
    ident = nc.dram_tensor("ident", [P, P], F32, kind="ExternalInput").ap()
    out = nc.dram_tensor("out", [CORE_BATCH, HID], F32, kind="ExternalOutput").ap()

    n_t = 2 * steps + 1  # distinct t values on the dt/2 grid

    with tile.TileContext(nc) as tc, ExitStack() as ctx:
        pers = ctx.enter_context(tc.tile_pool(name="pers", bufs=1))
        stage_pool = ctx.enter_context(tc.tile_pool(name="stage", bufs=3))
        psmm = ctx.enter_context(tc.tile_pool(name="psmm", bufs=5, space="PSUM"))
        pstr = ctx.enter_context(tc.tile_pool(name="pstr", bufs=2, space="PSUM"))

        # weights: [p, k, m*P+j] = W[k*P+p, m*P+j]
        w1s = pers.tile([P, KT, HID], FP16, tag="w1s")
        w2s = pers.tile([P, KT, HID], FP16, tag="w2s")
        w3s = pers.tile([P, KT, HID], FP16, tag="w3s")
        # activations, transposed: [p, m, b] = x[b, m*P+p]; one set per
        # 512-column batch chunk -- both chunks stay resident so the PE can
        # interleave them at layer granularity (hides drain latency)
        hT, hTb, acc, x0, x1 = [], [], [], [], []
        for c in range(chunks):
            hT_c = pers.tile([P, MT, NCHUNK], F32, tag=f"hT{c}", name=f"hT{c}")
            hTb_c = pers.tile([P, MT, NCHUNK], FP16, tag=f"hTb{c}", name=f"hTb{c}")
            acc_c = pers.tile([P, MT, NCHUNK], F32, tag=f"acc{c}", name=f"acc{c}")
            x0_c = pers.tile([P, MT, NCHUNK], FP16, tag=f"x0{c}", name=f"x0{c}")
            x1_c = pers.tile([P, MT, NCHUNK], FP16, tag=f"x1{c}", name=f"x1{c}")
            hT.append(hT_c); hTb.append(hTb_c); acc.append(acc_c)
            x0.append(x0_c); x1.append(x1_c)
        idt = pers.tile([P, P], F32, tag="idt")
        # per-partition bias columns: [p, m] = v[m*P+p]
        w1r = pers.tile([P, MT], FP16, tag="w1r")
        b1t = pers.tile([P, MT], F32, tag="b1t")
        b2t = pers.tile([P, MT], F32, tag="b2t")
        b3t = pers.tile([P, MT], F32, tag="b3t")
        b3dt = pers.tile([P, MT], F32, tag="b3dt")    # dt * b3
        b3h = pers.tile([P, MT], F32, tag="b3h")      # dt/2 * b3
        b1eff = pers.tile([P, MT, n_t], F32, tag="b1eff")  # b1 + t*W1[-1]

        dma = nc.sync.dma_start

        for ws, W in [(w1s, W1), (w2s, W2), (w3s, W3)]:
            for k in range(KT):
                dma(out=ws[:, k, :], in_=W[P * k : P * (k + 1), :])
        dma(out=idt[:], in_=ident)
        dma(out=w1r[:], in_=W1[HID : HID + 1, :].rearrange("o (m p) -> p (o m)", p=P))
        dma(out=b1t[:], in_=b1.rearrange("(m p) -> p m", p=P))
        dma(out=b2t[:], in_=b2.rearrange("(m p) -> p m", p=P))
        dma(out=b3t[:], in_=b3.rearrange("(m p) -> p m", p=P))

        nc.vector.tensor_scalar_mul(b3dt[:], b3t[:], DT)
        nc.vector.tensor_scalar_mul(b3h[:], b3t[:], DT / 2)
        for ti in range(n_t):
            nc.vector.scalar_tensor_tensor(
                b1eff[:, :, ti], w1r[:], ti * DT / 2, b1t[:], ALU.mult, ALU.add
            )

        def layer(src, ws, drain):
            """psum[m] = sum_k ws[k,m]^T @ src[k]; drain(ps, m) finishes it."""
            for m in range(MT):
                ps = psmm.tile([P, NCHUNK], F32, tag="ps")
                for k in range(KT):
                    nc.tensor.matmul(
                        ps[:],
                        ws[:, k, P * m : P * (m + 1)],
                        src[:, k, :],
                        start=(k == 0),
                        stop=(k == KT - 1),
                    )
                drain(ps, m)

        # ---- load all chunks, transposed via PE ----
        for c in range(chunks):
            rows0 = c * NCHUNK
            for bt in range(NCHUNK // P):
                stg = stage_pool.tile([P, HID], F32, tag="stg")
                dma(out=stg[:], in_=h_in[rows0 + P * bt : rows0 + P * (bt + 1), :])
                for j in range(MT):
                    pt = pstr.tile([P, P], F32, tag="pt")
                    nc.tensor.transpose(pt[:], stg[:, P * j : P * (j + 1)], idt[:])
                    nc.vector.tensor_copy(hT[c][:, j, P * bt : P * (bt + 1)], pt[:])
                    nc.vector.tensor_copy(hTb[c][:, j, P * bt : P * (bt + 1)], pt[:])

        # ---- RK4 steps, chunks interleaved at layer granularity ----
        def steps_body():
          for st in range(steps):
              for ev in range(4):
                  tidx = 2 * st + T_OFF[ev]
                  plans = []
                  for c in range(chunks):
                      srcs = [hTb[c], x0[c], x1[c], x0[c]]
                      d1s = [x0[c], x1[c], x0[c], x1[c]]
                      d2s = [x1[c], x0[c], x1[c], x0[c]]

                      def drain_tanh1(ps, m, ev=ev, tidx=tidx, d1s=d1s):
                          nc.scalar.activation(
                              d1s[ev][:, m, :], ps[:], AF.Tanh,
                              bias=b1eff[:, m, tidx : tidx + 1], scale=1.0,
                          )

                      def drain_tanh2(ps, m, ev=ev, d2s=d2s):
                          nc.scalar.activation(
                              d2s[ev][:, m, :], ps[:], AF.Tanh,
                              bias=b2t[:, m : m + 1], scale=1.0,
                          )

                      def drain_k(ps, m, ev=ev, c=c, d1s=d1s):
                          # ps = k_e - b3 (bias folded into the combines below)
                          if ev == 0:
                              # acc = h + (dt/6)*ps1   (b3 terms folded at ev3)
                              nc.vector.scalar_tensor_tensor(
                                  acc[c][:, m, :], ps[:], ACC_W[0], hT[c][:, m, :],
                                  ALU.mult, ALU.add,
                              )
                          elif ev == 3:
                              # hT = acc + (dt/6)*ps4 + dt*b3  -> new state
                              nc.scalar.activation(
                                  hT[c][:, m, :], ps[:], AF.Identity,
                                  bias=b3dt[:, m : m + 1], scale=ACC_W[3],
                              )
                              nc.vector.tensor_add(
                                  hT[c][:, m, :], hT[c][:, m, :], acc[c][:, m, :]
                              )
                              nc.vector.tensor_copy(
                                  hTb[c][:, m, :], hT[c][:, m, :]
                              )
                          else:
                              nc.vector.scalar_tensor_tensor(
                                  acc[c][:, m, :], ps[:], ACC_W[ev], acc[c][:, m, :],
                                  ALU.mult, ALU.add,
                              )
                          if ev < 3:
                              # h_tmp = h + c*(ps + b3), into d1s[ev]'s buffer
                              # (free again: layer 2 has consumed it)
                              ht = d1s[ev]
                              cb = b3h if ev < 2 else b3dt
                              nc.scalar.activation(
                                  ht[:, m, :], ps[:], AF.Identity,
                                  bias=cb[:, m : m + 1], scale=STEP_C[ev],
                              )
                              nc.vector.tensor_add(
                                  ht[:, m, :], ht[:, m, :], hT[c][:, m, :]
                              )

                      plans.append((srcs, d1s, d2s, drain_tanh1,
                                    drain_tanh2, drain_k))
                  # alternate chunks per layer: while chunk A's drains
                  # finish, the PE streams chunk B's matmuls -- no bubble
                  for srcs, _, _, dr1, _, _ in plans:
                      layer(srcs[ev], w1s, dr1)
                  for _, d1s, _, _, dr2, _ in plans:
                      layer(d1s[ev], w2s, dr2)
                  for _, _, d2s, _, _, dr3 in plans:
                      layer(d2s[ev], w3s, dr3)

        if reps == 1:
            steps_body()
        else:
            # timing mode: repeat the whole integration on-device so
            # kernel time dwarfs the host/RPC dispatch noise
            with tc.For_i(0, reps, 1):
                steps_body()

        # ---- store all chunks, transposed back ----
        for c in range(chunks):
            rows0 = c * NCHUNK
            for bt in range(NCHUNK // P):
                stg = stage_pool.tile([P, HID], F32, tag="stg")
                for j in range(MT):
                    pt = pstr.tile([P, P], F32, tag="pt")
                    nc.tensor.transpose(pt[:], hT[c][:, j, P * bt : P * (bt + 1)], idt[:])
                    nc.vector.tensor_copy(stg[:, P * j : P * (j + 1)], pt[:])
                dma(out=out[rows0 + P * bt : rows0 + P * (bt + 1), :], in_=stg[:])

    nc.compile()
    return nc


_NC_CACHE = {}


def get_nc(steps=STEPS, chunks=CHUNKS, reps=1):
    key = (steps, chunks, reps)
    if key not in _NC_CACHE:
        _NC_CACHE[key] = build_nc(steps, chunks, reps)
    return _NC_CACHE[key]


def make_in_maps(inputs):
    eye = np.eye(P, dtype=np.float32)
    full = {k: np.ascontiguousarray(np.asarray(v, dtype=np.float32))
            for k, v in inputs.items()}
    for w in ("W1", "W2", "W3"):
        full[w] = np.ascontiguousarray(full[w].astype(np.float16))
    in_maps = []
    for c in range(N_CORES):
        m = dict(full)
        m["h"] = np.ascontiguousarray(
            full["h"][c * CORE_BATCH : (c + 1) * CORE_BATCH]
        )
        m["ident"] = eye
        in_maps.append(m)
    return in_maps


def kernel(**inputs):
    nc = get_nc()
    in_maps = make_in_maps(inputs)
    res = run_bass_kernel_spmd(nc, in_maps, list(range(N_CORES)))
    return np.concatenate(
        [res.results[c]["out"] for c in range(N_CORES)], axis=0
    )



# revision 6
# speedup vs baseline: 27.6260x; 27.6260x over previous
"""Neural ODE layer (3-layer tanh MLP dynamics, RK4, 10 steps) on 8 trn2 cores.

Strategy: data-parallel over batch (8192/8 = 1024 rows per core), weights
replicated. Per core the batch is split into 2 chunks of 512 columns, both
SBUF-resident and interleaved at layer granularity. All activations live in
SBUF transposed ([hid on partitions, batch free]) so every matmul is
out^T = W^T @ x^T with the weight slice stationary -- the whole chain runs
without a single transpose.

Matmul operands are fp8 e4m3 driven in DoubleRow perf mode: each matmul
instruction contracts 256 rows (two 128-row k-slices packed along the free
dim of both operands) at 0.5 PE cycles per output element -- 4x the fp16
rate. The weights are pre-scaled by SW=2048 on the host so every entry sits
in e4m3's normal range (min normal 2^-6 would otherwise swallow half the
U(-1/32,1/32) mass as subnormals); the 1/SW is folded into the PSUM-drain
scales. DoubleRow caps the moving free at 2x256, so each 512-column PSUM
bank accumulates as two 256-column groups; the first matmul's start bit
zeroes the whole 2KB bank (PSUM zero-region granularity), the second group
accumulates into its pre-zeroed half with start=False. Matmuls are ordered
kp-outer/nh-inner so each stationary load serves 256 PE cycles of compute
and fast-weight-load stays hidden.

All bias handling is folded away from the hot loop: concat(h,t) @ W1 ==
h @ W1[:-1] + t*W1[-1], and the per-eval "+ c*b3" term of the RK4
half-steps is *exactly* equivalent to evaluating at t: the deficit of b3
in the carried state h equals t*b3 on RK4's time grid, so it linearizes
through layer 1 as t*(b3 @ W1[:-1]). Host precomputes
b1eff[ti] = b1 + ti*(dt/2)*(W1[-1] + b3 @ W1[:-1]); k3..k1 combines on DVE
never see b3, and the final output adds steps*dt*b3 once. RK4 state h and
the accumulator stay fp32; the combine is fused into DVE
scalar_tensor_tensor drains, tanh+bias into ACT drains.

Built as bacc.Bacc and finished with nc.compile() (splits multi-semaphore
waits into EventSemaphore instructions for walrus codegen).
"""

import sys

sys.path.insert(0, "/opt/trn_rl_repo")

import numpy as np
from contextlib import ExitStack

import concourse.bacc as bacc
import concourse.tile as tile
from concourse import mybir
from concourse.bass_utils import run_bass_kernel_spmd

HID = 1024
BATCH = 8192
N_CORES = 8
CORE_BATCH = BATCH // N_CORES  # 1024
DT = 0.1
STEPS = 10
P = 128
KT = HID // P  # 8 contraction tiles
KP = KT // 2   # 4 DoubleRow contraction pairs
MT = HID // P  # 8 output tiles
NCHUNK = 512   # batch columns per chunk (= one fp32 PSUM bank)
NH = 256       # DoubleRow moving cap: 2*NH <= 512
CHUNKS = CORE_BATCH // NCHUNK  # 2
SW = 2048.0    # host weight pre-scale into e4m3 normal range
N_T = 2 * STEPS + 1  # t grid (dt/2 units) for the default build

F32 = mybir.dt.float32
F8 = mybir.dt.float8e4
AF = mybir.ActivationFunctionType
ALU = mybir.AluOpType
DR = mybir.MatmulPerfMode.DoubleRow

# RK4: h' = h + dt/6*(k1 + 2k2 + 2k3 + k4)
ACC_W = [DT / 6, DT / 3, DT / 3, DT / 6]   # weight of k_e in the combine
STEP_C = [DT / 2, DT / 2, DT]              # h_tmp = h + c*k_e  (evals 0..2)
T_OFF = [0, 1, 1, 2]                       # t index offset (in dt/2 units)


def build_nc(steps=STEPS, chunks=CHUNKS, reps=1):
    nc = bacc.Bacc("TRN2", target_bir_lowering=False, debug=False)

    h_in = nc.dram_tensor("h", [CORE_BATCH, HID], F32, kind="ExternalInput").ap()
    W1 = nc.dram_tensor("W1q", [HID, HID], F8, kind="ExternalInput").ap()
    W2 = nc.dram_tensor("W2q", [HID, HID], F8, kind="ExternalInput").ap()
    W3 = nc.dram_tensor("W3q", [HID, HID], F8, kind="ExternalInput").ap()
    n_t = 2 * steps + 1
    b1e_d = nc.dram_tensor("b1eff", [HID, n_t], F32, kind="ExternalInput").ap()
    b2_d = nc.dram_tensor("b2v", [HID], F32, kind="ExternalInput").ap()
    b3_d = nc.dram_tensor("b3v", [HID], F32, kind="ExternalInput").ap()
    ident = nc.dram_tensor("ident", [P, P], F32, kind="ExternalInput").ap()
    out = nc.dram_tensor("out", [CORE_BATCH, HID], F32, kind="ExternalOutput").ap()

    with tile.TileContext(nc) as tc, ExitStack() as ctx:
        pers = ctx.enter_context(tc.tile_pool(name="pers", bufs=1))
        stage_pool = ctx.enter_context(tc.tile_pool(name="stage", bufs=3))
        psmm = ctx.enter_context(tc.tile_pool(name="psmm", bufs=6, space="PSUM"))
        pstr = ctx.enter_context(tc.tile_pool(name="pstr", bufs=2, space="PSUM"))

        # weights: [p, k, m*P+j] = W[k*P+p, m*P+j], pre-scaled by SW on host
        w1s = pers.tile([P, KT, HID], F8, tag="w1s")
        w2s = pers.tile([P, KT, HID], F8, tag="w2s")
        w3s = pers.tile([P, KT, HID], F8, tag="w3s")
        # activations, transposed: [p, m, b] = x[b, m*P+p]; one set per
        # 512-column batch chunk -- both chunks stay resident so the PE can
        # interleave them at layer granularity (hides drain latency)
        hT, hTb, acc, x0, x1 = [], [], [], [], []
        for c in range(chunks):
            hT.append(pers.tile([P, MT, NCHUNK], F32, tag=f"hT{c}", name=f"hT{c}"))
            hTb.append(pers.tile([P, MT, NCHUNK], F8, tag=f"hTb{c}", name=f"hTb{c}"))
            acc.append(pers.tile([P, MT, NCHUNK], F32, tag=f"acc{c}", name=f"acc{c}"))
            x0.append(pers.tile([P, MT, NCHUNK], F8, tag=f"x0{c}", name=f"x0{c}"))
            x1.append(pers.tile([P, MT, NCHUNK], F8, tag=f"x1{c}", name=f"x1{c}"))
        idt = pers.tile([P, P], F32, tag="idt")
        # per-partition bias columns: [p, m] = v[m*P+p]
        b1eff = pers.tile([P, MT, n_t], F32, tag="b1eff")
        b2t = pers.tile([P, MT], F32, tag="b2t")
        b3t = pers.tile([P, MT], F32, tag="b3t")
        b3fin = pers.tile([P, MT], F32, tag="b3fin")  # steps*dt * b3

        dma = nc.sync.dma_start

        for ws, W in [(w1s, W1), (w2s, W2), (w3s, W3)]:
            for k in range(KT):
                dma(out=ws[:, k, :], in_=W[P * k : P * (k + 1), :])
        dma(out=idt[:], in_=ident)
        dma(out=b1eff[:], in_=b1e_d.rearrange("(m p) t -> p m t", p=P))
        dma(out=b2t[:], in_=b2_d.rearrange("(m p) -> p m", p=P))
        dma(out=b3t[:], in_=b3_d.rearrange("(m p) -> p m", p=P))
        nc.vector.tensor_scalar_mul(b3fin[:], b3t[:], steps * DT)

        def layer(src, ws, drain):
            """psum[m] = sum_k ws[k,m]^T @ src[k] via DoubleRow; drain(ps, m).

            kp-outer / nh-inner: each stationary (2x128-row fp8) load feeds
            two 256-col moving passes (256 PE cycles) so fast-weight-load of
            the next slice stays hidden. start on the very first matmul
            zeroes the whole 2KB bank; the nh=1 group accumulates into its
            pre-zeroed half with start=False.
            """
            for m in range(MT):
                ps = psmm.tile([P, NCHUNK], F32, tag="ps")
                for kp in range(KP):
                    for nh in range(2):
                        nc.tensor.matmul(
                            ps[:, NH * nh : NH * (nh + 1)],
                            ws[:, 2 * kp : 2 * kp + 2, P * m : P * (m + 1)],
                            src[:, 2 * kp : 2 * kp + 2, NH * nh : NH * (nh + 1)],
                            start=(kp == 0 and nh == 0),
                            stop=(kp == KP - 1),
                            perf_mode=DR,
                            skip_group_check=True,
                        )
                drain(ps, m)

        # ---- load all chunks, transposed via PE ----
        for c in range(chunks):
            rows0 = c * NCHUNK
            for bt in range(NCHUNK // P):
                stg = stage_pool.tile([P, HID], F32, tag="stg")
                dma(out=stg[:], in_=h_in[rows0 + P * bt : rows0 + P * (bt + 1), :])
                for j in range(MT):
                    pt = pstr.tile([P, P], F32, tag="pt")
                    nc.tensor.transpose(pt[:], stg[:, P * j : P * (j + 1)], idt[:])
                    nc.vector.tensor_copy(hT[c][:, j, P * bt : P * (bt + 1)], pt[:])
                    nc.vector.tensor_copy(hTb[c][:, j, P * bt : P * (bt + 1)], pt[:])

        # ---- RK4 steps, chunks interleaved at layer granularity ----
        def steps_body():
          for st in range(steps):
              for ev in range(4):
                  tidx = 2 * st + T_OFF[ev]
                  plans = []
                  for c in range(chunks):
                      srcs = [hTb[c], x0[c], x1[c], x0[c]]
                      d1s = [x0[c], x1[c], x0[c], x1[c]]
                      d2s = [x1[c], x0[c], x1[c], x0[c]]

                      def drain_tanh1(ps, m, ev=ev, tidx=tidx, d1s=d1s):
                          nc.scalar.activation(
                              d1s[ev][:, m, :], ps[:], AF.Tanh,
                              bias=b1eff[:, m, tidx : tidx + 1], scale=1.0 / SW,
                          )

                      def drain_tanh2(ps, m, ev=ev, d2s=d2s):
                          nc.scalar.activation(
                              d2s[ev][:, m, :], ps[:], AF.Tanh,
                              bias=b2t[:, m : m + 1], scale=1.0 / SW,
                          )

                      def drain_k(ps, m, ev=ev, c=c, d1s=d1s):
                          # ps = SW * (x @ W3); b3 never appears here -- its
                          # deficit is folded into b1eff (t grid) + b3fin
                          if ev == 0:
                              nc.vector.scalar_tensor_tensor(
                                  acc[c][:, m, :], ps[:], ACC_W[0] / SW,
                                  hT[c][:, m, :], ALU.mult, ALU.add,
                              )
                          elif ev == 3:
                              nc.vector.scalar_tensor_tensor(
                                  hT[c][:, m, :], ps[:], ACC_W[3] / SW,
                                  acc[c][:, m, :], ALU.mult, ALU.add,
                              )
                              nc.vector.tensor_copy(
                                  hTb[c][:, m, :], hT[c][:, m, :]
                              )
                          else:
                              nc.vector.scalar_tensor_tensor(
                                  acc[c][:, m, :], ps[:], ACC_W[ev] / SW,
                                  acc[c][:, m, :], ALU.mult, ALU.add,
                              )
                          if ev < 3:
                              # h_tmp = h + c*k_e, straight to fp8 moving
                              # input for the next eval's layer 1
                              nc.vector.scalar_tensor_tensor(
                                  d1s[ev][:, m, :], ps[:], STEP_C[ev] / SW,
                                  hT[c][:, m, :], ALU.mult, ALU.add,
                              )

                      plans.append((srcs, d1s, d2s, drain_tanh1,
                                    drain_tanh2, drain_k))
                  # alternate chunks per layer: while chunk A's drains
                  # finish, the PE streams chunk B's matmuls -- no bubble
                  for srcs, _, _, dr1, _, _ in plans:
                      layer(srcs[ev], w1s, dr1)
                  for _, d1s, _, _, dr2, _ in plans:
                      layer(d1s[ev], w2s, dr2)
                  for _, _, d2s, _, _, dr3 in plans:
                      layer(d2s[ev], w3s, dr3)

        if reps == 1:
            steps_body()
        else:
            # timing mode: repeat the whole integration on-device
            with tc.For_i(0, reps, 1):
                steps_body()

        # ---- add the b3 deficit, store all chunks transposed back ----
        for c in range(chunks):
            for m in range(MT):
                nc.scalar.activation(
                    hT[c][:, m, :], hT[c][:, m, :], AF.Identity,
                    bias=b3fin[:, m : m + 1], scale=1.0,
                )
        for c in range(chunks):
            rows0 = c * NCHUNK
            for bt in range(NCHUNK // P):
                stg = stage_pool.tile([P, HID], F32, tag="stg")
                for j in range(MT):
                    pt = pstr.tile([P, P], F32, tag="pt")
                    nc.tensor.transpose(pt[:], hT[c][:, j, P * bt : P * (bt + 1)], idt[:])
                    nc.vector.tensor_copy(stg[:, P * j : P * (j + 1)], pt[:])
                dma(out=out[rows0 + P * bt : rows0 + P * (bt + 1), :], in_=stg[:])

    nc.compile()
    return nc


_NC_CACHE = {}


def get_nc(steps=STEPS, chunks=CHUNKS, reps=1):
    key = (steps, chunks, reps)
    if key not in _NC_CACHE:
        _NC_CACHE[key] = build_nc(steps, chunks, reps)
    return _NC_CACHE[key]


def make_in_maps(inputs, steps=STEPS):
    import ml_dtypes

    eye = np.eye(P, dtype=np.float32)
    f = {k: np.asarray(v, dtype=np.float32) for k, v in inputs.items()}
    W1f, b1 = f["W1"], f["b1"]
    b3 = f["b3"]
    w1r_eff = W1f[HID] + b3 @ W1f[:HID]
    n_t = 2 * steps + 1
    b1eff = np.stack(
        [b1 + ti * (DT / 2) * w1r_eff for ti in range(n_t)], axis=1
    ).astype(np.float32)  # [HID, n_t]
    q8 = lambda w: np.ascontiguousarray(
        (w * SW).astype(ml_dtypes.float8_e4m3)
    )
    base = {
        "W1q": q8(W1f[:HID]),
        "W2q": q8(f["W2"]),
        "W3q": q8(f["W3"]),
        "b1eff": np.ascontiguousarray(b1eff),
        "b2v": np.ascontiguousarray(f["b2"]),
        "b3v": np.ascontiguousarray(b3),
        "ident": eye,
    }
    in_maps = []
    for c in range(N_CORES):
        m = dict(base)
        m["h"] = np.ascontiguousarray(
            f["h"][c * CORE_BATCH : (c + 1) * CORE_BATCH]
        )
        in_maps.append(m)
    return in_maps


def kernel(**inputs):
    nc = get_nc()
    in_maps = make_in_maps(inputs)
    res = run_bass_kernel_spmd(nc, in_maps, list(range(N_CORES)))
    return np.concatenate(
        [res.results[c]["out"] for c in range(N_CORES)], axis=0
    )
